# revision 1
# baseline (speedup 1.0000x reference)
"""Gaussian-kernel attention for Trainium2 (Bass/Tile), 8-core data-parallel.

Computes out = x + K @ x with K = exp(-r * d2), d2[t,s] = ||x_t - x_s||^2,
per batch.  Decomposition used on-chip:

    d2 = sq_t + sq_s - 2*G          (G = X X^T, sq = rowwise |x|^2)
    K  = e_t * exp(2r*G) * e_s      (e_i = exp(-r*sq_i))
    out[t] = x[t] + e_t * sum_s exp(2r*G)[s,t] * (e_s * x[s])

so the only T^2-scale elementwise op is a single fused ACT exp per tile
(scale immediate = 2r), and the diagonal factors fold into a cheap O(T*C)
pre-scale (Y = e_s * x) and a per-row epilogue scale (e_t).

Sharding: pure data-parallel over batch B=32 -> 4 batches per core x 8 cores.
"""

import os
import sys

import numpy as np

sys.path.insert(0, "/opt/trn_rl_repo")

import concourse.bass as bass
import concourse.tile as tile
from concourse import bacc, mybir
from concourse.bass_utils import run_bass_kernel_spmd

FP32 = mybir.dt.float32
BF16 = mybir.dt.bfloat16

B, T, C = 32, 2048, 64
N_CORES = 8
BPC = B // N_CORES  # batches per core

# Stashed by kernel() for the test harness (exec time etc.)
LAST_RESULTS = None


def _body(ctx, tc, out_ap, x_ap, r, bpc, t, dbg=False):
    """Emit the per-core kernel IR.

    out_ap/x_ap: DRAM APs of shape [bpc, t, C].
    r: python float (r_sigma value, baked as immediates).
    """
    nc = tc.nc

    def dump(name, sb_ap, dt=None):
        if not dbg:
            return
        d = nc.dram_tensor(
            name, list(sb_ap.shape), dt or sb_ap.dtype, kind="ExternalOutput"
        ).ap()
        nc.sync.dma_start(out=d, in_=sb_ap)
    nt = t // 128          # 128-row s/t blocks
    tblk = min(1024, t)    # t-block width processed per G tile (<= 2 psum banks)
    ntb = t // tblk        # outer t-block count
    kpb = tblk // 128      # 128-col sub-blocks per t-block

    exp2r = 2.0 * r

    # SBUF pools (bufs=2 for cross-batch pipelining)
    xpool = ctx.enter_context(tc.tile_pool(name="x32", bufs=2))
    xxpool = ctx.enter_context(tc.tile_pool(name="xx", bufs=2))
    sqpool = ctx.enter_context(tc.tile_pool(name="sq", bufs=2))
    ypool = ctx.enter_context(tc.tile_pool(name="yb", bufs=2))
    xbpool = ctx.enter_context(tc.tile_pool(name="xbp", bufs=2))
    xtpool = ctx.enter_context(tc.tile_pool(name="xt", bufs=2))
    apool = ctx.enter_context(tc.tile_pool(name="a0", bufs=3))
    opool = ctx.enter_context(tc.tile_pool(name="osb", bufs=2))
    # PSUM: G = [128, tblk] fp32 (tblk/512 banks) x2 bufs; P = [128, 512] x2 bufs
    gpool = ctx.enter_context(tc.tile_pool(name="gps", bufs=2, space="PSUM"))
    ppool = ctx.enter_context(tc.tile_pool(name="pps", bufs=2, space="PSUM"))
    # DRAM scratch for the bf16 transpose round-trip
    dpool = ctx.enter_context(tc.tile_pool(name="dsc", bufs=2, space="DRAM"))

    for b in range(bpc):
        xb_dram = x_ap[b].rearrange("(k p) c -> p k c", p=128)   # [128, nt, C]
        ob_dram = out_ap[b].rearrange("(k p) c -> p k c", p=128)

        # ---- prologue: load, row stats, Y = e_s * x (bf16), X^T (bf16) ----
        x32 = xpool.tile([128, nt, C], FP32)
        nc.sync.dma_start(out=x32[:], in_=xb_dram)

        xx = xxpool.tile([128, nt, C], FP32)
        nc.vector.tensor_mul(xx[:], x32[:], x32[:])
        sq = sqpool.tile([128, nt], FP32, tag="sq")
        nc.vector.tensor_reduce(
            sq[:], xx[:], axis=mybir.AxisListType.X, op=mybir.AluOpType.add
        )
        ev = sqpool.tile([128, nt], FP32, tag="ev")
        nc.scalar.activation(
            ev[:], sq[:], mybir.ActivationFunctionType.Exp, scale=-r
        )

        yb = ypool.tile([128, nt, C], BF16)
        for k in range(nt):
            nc.vector.tensor_scalar_mul(yb[:, k], x32[:, k], ev[:, k : k + 1])

        # bf16 copy of x written TWICE side by side (cols 0:C and C:2C) so a
        # single full-width DMA-xbar transpose yields X^T duplicated on both
        # partition halves -- mm1 then row-packs two K=64 matmuls into the
        # PE's 128 rows (tile_position row groups 0-1 / 2-3).
        xbp = xbpool.tile([128, nt, 2 * C], BF16)
        nc.vector.tensor_copy(xbp[:, :, 0:C], x32[:])
        nc.vector.tensor_copy(xbp[:, :, C : 2 * C], x32[:])
        xbd = dpool.tile([t, 2 * C], BF16)
        nc.sync.dma_start(out=xbd.rearrange("(k p) c -> p k c", p=128), in_=xbp[:])
        xt = xtpool.tile([128, t], BF16)
        nc.sync.dma_start_transpose(out=xt[:], in_=xbd[:])
        # xt[c, tt] = xt[64+c, tt] = x[tt, c] for c < 64.
        if dbg and b == 0:
            dump("dbg_sq", sq[:])
            dump("dbg_ev", ev[:])
            dump("dbg_yb", yb[:])
            dump("dbg_xt", xt[:])

        # ---- main: per t-block, accumulate over all s blocks ----
        # mm2 keeps A0 as the MOVING operand (N=512) with Yb_s stationary
        # (64-col weight load, once per s) so the PE isn't throttled by a
        # 128-col LDWEIGHTS stream per 64-col matmul.  Result lands
        # transposed (outT[c, t]) in PSUM; the fixup transpose runs as bf16
        # DMA-xbar ops on the otherwise-idle DMA engines.
        for ti in range(ntb):
            p_ps = ppool.tile([C, tblk], FP32)

            # Software-pipelined emission: mm1(s+1) is issued BEFORE mm2(s)
            # so the PE's strict-FIFO queue never has a blocked mm2 at its
            # head starving ready mm1 work behind it (head-of-line stalls
            # keep HAM throttled at 1.2 GHz).  G is double-buffered.
            def mm1(s, h):
                g = gpool.tile([128, tblk], FP32, name="g_ps", tag="g")
                base = 64 * h
                for n in range(tblk // 512):
                    nc.tensor.matmul(
                        g[:, n * 512 : (n + 1) * 512],
                        lhsT=xt[base : base + 64, s * 128 : (s + 1) * 128],
                        rhs=xt[
                            base : base + 64,
                            ti * tblk + n * 512 : ti * tblk + (n + 1) * 512,
                        ],
                        start=True,
                        stop=True,
                    )
                return g

            g_cur = mm1(0, 0)
            for s in range(nt):
                g_next = mm1(s + 1, (s + 1) % 2) if s + 1 < nt else None
                a0 = apool.tile([128, tblk], BF16)
                nc.scalar.activation(
                    a0[:], g_cur[:], mybir.ActivationFunctionType.Exp, scale=exp2r
                )
                if dbg and b == 0 and ti == 0 and s == 0:
                    gsb = xxpool.tile([128, tblk], FP32, tag="gdump")
                    nc.vector.tensor_copy(gsb[:], g_cur[:])
                    dump("dbg_g00", gsb[:])
                    dump("dbg_a00", a0[:])
                for n in range(tblk // 512):
                    # each 512-slice is its own PSUM bank -> start=True
                    # at s==0 clears exactly that bank
                    nc.tensor.matmul(
                        p_ps[:, n * 512 : (n + 1) * 512],
                        lhsT=yb[:, s],
                        rhs=a0[:, n * 512 : (n + 1) * 512],
                        start=(s == 0),
                        stop=(s == nt - 1),
                        skip_group_check=True,
                    )
                g_cur = g_next
            # ---- epilogue: transpose outT via bf16 DMA-xbar, then
            # out = outT.T * e_t + x ----
            ot = opool.tile([C, tblk], BF16, tag="ot")
            nc.vector.tensor_copy(ot[:], p_ps[:])
            tr = opool.tile([128, kpb, C], BF16, tag="tr")
            for j in range(kpb):
                nc.sync.dma_start_transpose(
                    out=tr[:, j], in_=ot[:, j * 128 : (j + 1) * 128]
                )
            osb = opool.tile([128, kpb, C], FP32, tag="osb")
            for j in range(kpb):
                k = ti * kpb + j
                nc.vector.scalar_tensor_tensor(
                    osb[:, j],
                    in0=tr[:, j],
                    scalar=ev[:, k : k + 1],
                    in1=x32[:, k],
                    op0=mybir.AluOpType.mult,
                    op1=mybir.AluOpType.add,
                )
            nc.sync.dma_start(
                out=ob_dram[:, ti * kpb : (ti + 1) * kpb], in_=osb[:]
            )


def build(r, bpc=BPC, t=T, dbg=False):
    """Build + compile the Bass module for one core's shard."""
    from contextlib import ExitStack

    nc = bacc.Bacc(
        "TRN2", target_bir_lowering=False, debug=False, num_devices=N_CORES
    )
    x_ap = nc.dram_tensor("x", [bpc, t, C], FP32, kind="ExternalInput").ap()
    out_ap = nc.dram_tensor("out", [bpc, t, C], FP32, kind="ExternalOutput").ap()
    with tile.TileContext(nc) as tc:
        with ExitStack() as ctx:
            _body(ctx, tc, out_ap, x_ap, r, bpc, t, dbg=dbg)
    nc.compile()
    return nc


def kernel(x, r_sigma):
    global LAST_RESULTS
    x = np.ascontiguousarray(np.asarray(x, dtype=np.float32))
    r = float(np.asarray(r_sigma).reshape(-1)[0])
    assert x.shape == (B, T, C), x.shape

    nc = build(r)
    in_maps = [
        {"x": np.ascontiguousarray(x[i * BPC : (i + 1) * BPC])}
        for i in range(N_CORES)
    ]
    trace = bool(int(os.environ.get("KERNEL_TRACE", "0")))
    res = run_bass_kernel_spmd(
        nc, in_maps, core_ids=list(range(N_CORES)), trace=trace
    )
    LAST_RESULTS = res
    out = np.concatenate([res.results[i]["out"] for i in range(N_CORES)], axis=0)
    return out.astype(np.float32)



# revision 4
# speedup vs baseline: 1.2832x; 1.2832x over previous
"""Gaussian-kernel attention for Trainium2 (Bass/Tile), 8-core data-parallel.

Computes out = x + K @ x with K = exp(-r * d2), d2[t,s] = ||x_t - x_s||^2,
per batch.  Decomposition used on-chip:

    d2 = sq_t + sq_s - 2*G          (G = X X^T, sq = rowwise |x|^2)
    K  = e_t * exp(2r*G) * e_s      (e_i = exp(-r*sq_i))
    out[t] = x[t] + e_t * sum_s exp(2r*G)[s,t] * (e_s * x[s])

Performance architecture (all-bf16 matmuls; fp8 was tried and rejected —
its quantization noise alone exceeds the 2e-2 error budget):

  * mm1 (G = X X^T, K=64 contraction) runs as CONCURRENT dual row-tile
    pairs: two s-blocks issue back-to-back into PE row groups at
    tile_position (0,0) and (64,0); the duplicated x^T layout (xt) feeds
    both halves, so a pair of 512-col matmuls spans ~one matmul time.
  * The T^2-sized exp splits across TWO engines working in parallel:
      - ACT pairs: true exp (scale=2r) -> bf16.
      - DVE pairs: Schraudolph bit-trick exp: i16 = int16(G*(2r*c1)+c2)
        reinterpreted as bf16 IS approximately exp(2r*G) (~1.5% rms);
        one DVE tensor_scalar per pair.
    Pairs containing diagonal blocks (largest K values) are forced to
    ACT (exact exp).
  * mm2 (M=64) also runs as CONCURRENT dual col-tile pairs: s-block 2j
    accumulates into partitions 0:64 of the PSUM bank (tile (0,0)),
    s-block 2j+1 into partitions 64:128 (tile (0,64)).  The two halves
    are copied out separately (ACT copies half A, DVE half B — each
    engine stays at one partition base, no cross-lane moves), transposed
    with ONE whole-batch DMA-xbar transpose each (the per-128-block
    variant costs ~1.2us fixed overhead apiece), and merged by chained
    scalar_tensor_tensor passes that also apply e_t and the +x residual.
  * GpSimd/Pool takes only big contiguous 1-input copies (the x^T dup
    staging) — its elementwise path is ~2x slower than DVE with heavy
    small-op overhead.

Sharding: pure data-parallel over batch B=32 -> 4 batches per core x 8 cores.
"""

import os
import sys

import numpy as np

sys.path.insert(0, "/opt/trn_rl_repo")

import concourse.bass as bass
import concourse.tile as tile
from concourse import bacc, mybir
from concourse.bass_utils import run_bass_kernel_spmd

FP32 = mybir.dt.float32
BF16 = mybir.dt.bfloat16
I16 = mybir.dt.int16

B, T, C = 32, 2048, 64
N_CORES = 8
BPC = B // N_CORES  # batches per core

TB = 512            # t-block width (one PSUM bank of mm2 accumulation)

# Schraudolph exp-as-bf16-bits constants:  bf16_bits(z*SCHRAU_C1 + SCHRAU_C2)
# ~= exp(z).  c1 = 2^7/ln2; c2 = 127*2^7 - 7.42 (minimax shift) + 0.5
# (float->int truncation in the convert).
SCHRAU_C1 = 128.0 / 0.6931471805599453
SCHRAU_C2 = 16256.0 - 7.42 + 0.5

# Fraction of non-diagonal pairs whose exp runs on ACT (rest on DVE).
# DVE also carries the prologue/epilogue elementwise work, so ACT takes
# the bigger share; diagonal pairs are forced to ACT on top of this.
ACT_NONDIAG_FRAC = 0.58

# Stashed by kernel() for the test harness (exec time etc.)
LAST_RESULTS = None


def _body(ctx, tc, out_ap, x_ap, r, bpc, t, dbg=False):
    """Emit the per-core kernel IR.

    out_ap/x_ap: DRAM APs of shape [bpc, t, C].
    r: python float (r_sigma value, baked as immediates).
    """
    nc = tc.nc

    def dump(name, sb_ap, dt=None):
        if not dbg:
            return
        d = nc.dram_tensor(
            name, list(sb_ap.shape), dt or sb_ap.dtype, kind="ExternalOutput"
        ).ap()
        nc.sync.dma_start(out=d, in_=sb_ap)

    nt = t // 128          # 128-row s/t blocks
    ntb = t // TB
    npair = nt // 2

    exp2r = 2.0 * r

    # SBUF pools (bufs=2 for cross-batch pipelining)
    xpool = ctx.enter_context(tc.tile_pool(name="x32", bufs=2))
    xxpool = ctx.enter_context(tc.tile_pool(name="xx", bufs=2))
    sqpool = ctx.enter_context(tc.tile_pool(name="sq", bufs=2))
    ypool = ctx.enter_context(tc.tile_pool(name="yb", bufs=2))
    xbpool = ctx.enter_context(tc.tile_pool(name="xbp", bufs=2))
    xtpool = ctx.enter_context(tc.tile_pool(name="xt", bufs=2))
    apool = ctx.enter_context(tc.tile_pool(name="a0", bufs=3))
    ipool = ctx.enter_context(tc.tile_pool(name="i16", bufs=3))
    otpool = ctx.enter_context(tc.tile_pool(name="otb", bufs=2))
    trpool = ctx.enter_context(tc.tile_pool(name="trb", bufs=2))
    opool = ctx.enter_context(tc.tile_pool(name="osb", bufs=2))
    # PSUM: g2 = [128, 2, TB] fp32 (2 banks) x2 bufs; p = [128, TB] x2 bufs
    gpool = ctx.enter_context(tc.tile_pool(name="gps", bufs=2, space="PSUM"))
    ppool = ctx.enter_context(tc.tile_pool(name="pps", bufs=2, space="PSUM"))
    # DRAM scratch for the bf16 transpose round-trip
    dpool = ctx.enter_context(tc.tile_pool(name="dsc", bufs=2, space="DRAM"))

    act_credit = 0.0  # fractional round-robin for non-diagonal pair owners

    for b in range(bpc):
        xb_dram = x_ap[b].rearrange("(k p) c -> p k c", p=128)   # [128, nt, C]
        ob_dram = out_ap[b].rearrange("(k p) c -> p k c", p=128)

        # ---- prologue: load, row stats, Y = e_s * x (bf16), X^T dup ----
        x32 = xpool.tile([128, nt, C], FP32)
        nc.sync.dma_start(out=x32[:], in_=xb_dram)

        xx = xxpool.tile([128, nt, C], FP32)
        nc.vector.tensor_mul(xx[:], x32[:], x32[:])
        sq = sqpool.tile([128, nt], FP32, tag="sq")
        nc.vector.tensor_reduce(
            sq[:], xx[:], axis=mybir.AxisListType.X, op=mybir.AluOpType.add
        )
        ev = sqpool.tile([128, nt], FP32, tag="ev")
        nc.scalar.activation(
            ev[:], sq[:], mybir.ActivationFunctionType.Exp, scale=-r
        )

        yb = ypool.tile([128, nt, C], BF16)
        for k in range(nt):
            nc.vector.tensor_scalar_mul(yb[:, k], x32[:, k], ev[:, k : k + 1])

        # bf16 copy of x written TWICE side by side (cols 0:C and C:2C) so a
        # single full-width DMA-xbar transpose yields X^T duplicated on both
        # partition halves -- mm1 streams two s-blocks concurrently through
        # PE row groups (0,0)/(64,0).
        xbp = xbpool.tile([128, nt, 2 * C], BF16)
        nc.gpsimd.tensor_copy(xbp[:, :, 0:C], x32[:])
        nc.gpsimd.tensor_copy(xbp[:, :, C : 2 * C], x32[:])
        xbd = dpool.tile([t, 2 * C], BF16)
        nc.sync.dma_start(out=xbd.rearrange("(k p) c -> p k c", p=128), in_=xbp[:])
        xt = xtpool.tile([128, t], BF16)
        nc.sync.dma_start_transpose(out=xt[:], in_=xbd[:])
        # xt[c, tt] = xt[64+c, tt] = x[tt, c] for c < 64.
        if dbg and b == 0:
            dump("dbg_sq", sq[:])
            dump("dbg_ev", ev[:])
            dump("dbg_yb", yb[:])
            dump("dbg_xt", xt[:])

        # ---- main loop: flat (ti, j) steps, software-pipelined mm1 ----
        # otb partitions 0:64 hold the s-even half of out^T; partitions
        # 64:128 the s-odd half (same-base engine copies only).
        otb = otpool.tile([128, t], BF16)

        steps = [(ti, j) for ti in range(ntb) for j in range(npair)]

        owners = []
        for ti, j in steps:
            if j in (2 * ti, 2 * ti + 1):
                owners.append("act")       # diagonal pair: exact exp
            else:
                act_credit += ACT_NONDIAG_FRAC
                if act_credit >= 1.0:
                    act_credit -= 1.0
                    owners.append("act")
                else:
                    owners.append("dve")

        def mm1(step):
            """Concurrent dual row-tile pair: G for s-blocks 2j, 2j+1."""
            ti, j = steps[step]
            g2 = gpool.tile([128, 2, TB], FP32, name="g_ps", tag="g")
            for i in range(2):
                base = 64 * i
                s = 2 * j + i
                nc.tensor.matmul(
                    g2[:, i],
                    lhsT=xt[base : base + 64, s * 128 : (s + 1) * 128],
                    rhs=xt[base : base + 64, ti * TB : (ti + 1) * TB],
                    start=True,
                    stop=True,
                )
            return g2

        p_ps = None
        g_cur = mm1(0)
        for step, (ti, j) in enumerate(steps):
            if j == 0:
                p_ps = ppool.tile([128, TB], FP32, tag="p")
            g_next = mm1(step + 1) if step + 1 < len(steps) else None

            if owners[step] == "act":
                a0t = apool.tile([128, 2, TB], BF16)
                nc.scalar.activation(
                    a0t[:], g_cur[:], mybir.ActivationFunctionType.Exp,
                    scale=exp2r,
                )
                a0 = a0t[:]
            else:
                i16 = ipool.tile([128, 2, TB], I16)
                nc.vector.tensor_scalar(
                    i16[:],
                    g_cur[:],
                    exp2r * SCHRAU_C1,
                    SCHRAU_C2,
                    op0=mybir.AluOpType.mult,
                    op1=mybir.AluOpType.add,
                )
                a0 = i16[:].bitcast(BF16)
            if dbg and b == 0 and step == 0:
                gsb = xxpool.tile([128, 2, TB], FP32, tag="gdump")
                nc.vector.tensor_copy(gsb[:], g_cur[:])
                dump("dbg_g00", gsb[:])

            # mm2: concurrent dual col-tile pair -> partition halves of p_ps
            for i in range(2):
                nc.tensor.matmul(
                    p_ps[64 * i : 64 * i + 64, :],
                    lhsT=yb[:, 2 * j + i],
                    rhs=a0[:, i],
                    start=(j == 0),
                    stop=(j == npair - 1),
                    tile_position=(0, 64 * i),
                    skip_group_check=True,
                )
            g_cur = g_next

            if j == npair - 1:
                # copy the two accumulator halves out at the SAME partition
                # base (ACT takes half A, DVE half B)
                nc.scalar.activation(
                    otb[0:64, ti * TB : (ti + 1) * TB], p_ps[0:64, :],
                    mybir.ActivationFunctionType.Copy,
                )
                nc.vector.tensor_copy(
                    otb[64:128, ti * TB : (ti + 1) * TB], p_ps[64:128, :]
                )

        # ---- epilogue: one whole-batch transpose per half, then
        # out = (trA + trB summed) * e_t + x via chained STTs on DVE ----
        trb = trpool.tile([128, 2, nt, C], BF16)
        nc.sync.dma_start_transpose(out=trb[:, 0], in_=otb[0:64, :])
        nc.sync.dma_start_transpose(out=trb[:, 1], in_=otb[64:128, :])
        if dbg and b == 0:
            dump("dbg_otb", otb[:])
            dump("dbg_trb", trb[:])
        o1 = opool.tile([128, nt, C], FP32, tag="o1")
        osb = opool.tile([128, nt, C], FP32, tag="osb")
        for k in range(nt):
            nc.vector.scalar_tensor_tensor(
                o1[:, k],
                in0=trb[:, 0, k],
                scalar=ev[:, k : k + 1],
                in1=x32[:, k],
                op0=mybir.AluOpType.mult,
                op1=mybir.AluOpType.add,
            )
            nc.vector.scalar_tensor_tensor(
                osb[:, k],
                in0=trb[:, 1, k],
                scalar=ev[:, k : k + 1],
                in1=o1[:, k],
                op0=mybir.AluOpType.mult,
                op1=mybir.AluOpType.add,
            )
        nc.sync.dma_start(out=ob_dram, in_=osb[:])


def build(r, bpc=BPC, t=T, dbg=False):
    """Build + compile the Bass module for one core's shard."""
    from contextlib import ExitStack

    nc = bacc.Bacc(
        "TRN2", target_bir_lowering=False, debug=False, num_devices=N_CORES
    )
    x_ap = nc.dram_tensor("x", [bpc, t, C], FP32, kind="ExternalInput").ap()
    out_ap = nc.dram_tensor("out", [bpc, t, C], FP32, kind="ExternalOutput").ap()
    with tile.TileContext(nc) as tc:
        with ExitStack() as ctx:
            _body(ctx, tc, out_ap, x_ap, r, bpc, t, dbg=dbg)
    nc.compile()
    return nc


def kernel(x, r_sigma):
    global LAST_RESULTS
    x = np.ascontiguousarray(np.asarray(x, dtype=np.float32))
    r = float(np.asarray(r_sigma).reshape(-1)[0])
    assert x.shape == (B, T, C), x.shape

    nc = build(r)
    in_maps = [
        {"x": np.ascontiguousarray(x[i * BPC : (i + 1) * BPC])}
        for i in range(N_CORES)
    ]
    trace = bool(int(os.environ.get("KERNEL_TRACE", "0")))
    res = run_bass_kernel_spmd(
        nc, in_maps, core_ids=list(range(N_CORES)), trace=trace
    )
    LAST_RESULTS = res
    out = np.concatenate([res.results[i]["out"] for i in range(N_CORES)], axis=0)
    return out.astype(np.float32)


# revision 5
# speedup vs baseline: 1.7152x; 1.3366x over previous
"""Gaussian-kernel attention for Trainium2 (Bass/Tile), 8-core data-parallel.

Computes out = x + K @ x with K = exp(-r * d2), d2[t,s] = ||x_t - x_s||^2,
per batch.  Decomposition used on-chip:

    d2 = sq_t + sq_s - 2*G          (G = X X^T, sq = rowwise |x|^2)
    K  = e_t * exp(2r*G) * e_s      (e_i = exp(-r*sq_i))
    out[t] = x[t] + e_t * sum_s exp(2r*G)[s,t] * (e_s * x[s])

Performance architecture (all-bf16 matmuls; fp8 was tried and rejected —
its quantization noise alone exceeds the 2e-2 error budget):

  * mm1 (G = X X^T, K=64 contraction) runs as CONCURRENT dual row-tile
    pairs: two s-blocks issue back-to-back into PE row groups at
    tile_position (0,0) and (64,0); the duplicated x^T layout (xt) feeds
    both halves, so a pair of 512-col matmuls spans ~one matmul time.
    mm1 runs TWO steps ahead of mm2 (g2 triple-buffered) so the exp
    latency never blocks the PE's strict-FIFO queue head — PE-idle
    micro-gaps reset the HAM clock gate (1.2 vs 2.4 GHz).
  * The T^2-sized exp splits across TWO engines working in parallel:
      - ACT pairs: true exp (scale=2r) -> bf16.
      - DVE pairs: Schraudolph bit-trick exp: i16 = int16(G*(2r*c1)+c2)
        reinterpreted as bf16 IS approximately exp(2r*G) (~1.5% rms);
        one DVE tensor_scalar per pair.
    Pairs containing diagonal blocks (largest K values) are forced to
    ACT (exact exp).
  * mm2 (M=64) also runs as CONCURRENT dual col-tile pairs: s-block 2j
    accumulates into partitions 0:64 of the PSUM bank (tile (0,0)),
    s-block 2j+1 into partitions 64:128 (tile (0,64)).  Each t-block's
    [128, TB] accumulator is copied out in ONE op (halves stay in their
    partition ranges), the two halves are transposed with one
    whole-batch DMA-xbar transpose each, and merged by big elementwise
    ops using stride-0 broadcast APs for the e_t scale (per-element ops
    on DVE cost ~1ns/row, but per-INSTRUCTION overhead is ~200-400ns,
    so everything is batched into [128, 1024]-sized ops).

Sharding: pure data-parallel over batch B=32 -> 4 batches per core x 8 cores.
"""

import os
import sys

import numpy as np

sys.path.insert(0, "/opt/trn_rl_repo")

import concourse.bass as bass
import concourse.tile as tile
from concourse import bacc, mybir
from concourse.bass_utils import run_bass_kernel_spmd

FP32 = mybir.dt.float32
BF16 = mybir.dt.bfloat16
I16 = mybir.dt.int16

B, T, C = 32, 2048, 64
N_CORES = 8
BPC = B // N_CORES  # batches per core

TB = 512            # t-block width (one PSUM bank of mm2 accumulation)

# Schraudolph exp-as-bf16-bits constants:  bf16_bits(z*SCHRAU_C1 + SCHRAU_C2)
# ~= exp(z).  c1 = 2^7/ln2; c2 = 127*2^7 - 7.42 (minimax shift) + 0.5
# (float->int truncation in the convert).
SCHRAU_C1 = 128.0 / 0.6931471805599453
SCHRAU_C2 = 16256.0 - 7.42 + 0.5

# Fraction of non-diagonal pairs whose exp runs on ACT (rest on DVE).
# DVE also carries the prologue/epilogue elementwise work, so ACT takes
# the bigger share; diagonal pairs are forced to ACT on top of this.
ACT_NONDIAG_FRAC = 0.54

# Stashed by kernel() for the test harness (exec time etc.)
LAST_RESULTS = None


def _body(ctx, tc, out_ap, x_ap, r, bpc, t, dbg=False):
    """Emit the per-core kernel IR.

    out_ap/x_ap: DRAM APs of shape [bpc, t, C].
    r: python float (r_sigma value, baked as immediates).
    """
    nc = tc.nc

    def dump(name, sb_ap, dt=None):
        if not dbg:
            return
        d = nc.dram_tensor(
            name, list(sb_ap.shape), dt or sb_ap.dtype, kind="ExternalOutput"
        ).ap()
        nc.sync.dma_start(out=d, in_=sb_ap)

    nt = t // 128          # 128-row s/t blocks
    ntb = t // TB
    npair = nt // 2

    exp2r = 2.0 * r

    # SBUF pools (bufs=2 for cross-batch pipelining)
    xpool = ctx.enter_context(tc.tile_pool(name="x32", bufs=2))
    xxpool = ctx.enter_context(tc.tile_pool(name="xx", bufs=2))
    sqpool = ctx.enter_context(tc.tile_pool(name="sq", bufs=2))
    ypool = ctx.enter_context(tc.tile_pool(name="yb", bufs=2))
    xbpool = ctx.enter_context(tc.tile_pool(name="xb16", bufs=2))
    xtpool = ctx.enter_context(tc.tile_pool(name="xt", bufs=2))
    apool = ctx.enter_context(tc.tile_pool(name="a0", bufs=4))
    ipool = ctx.enter_context(tc.tile_pool(name="i16", bufs=4))
    otpool = ctx.enter_context(tc.tile_pool(name="otb", bufs=2))
    trpool = ctx.enter_context(tc.tile_pool(name="trb", bufs=2))
    opool = ctx.enter_context(tc.tile_pool(name="osb", bufs=2))
    # PSUM (8 banks total): g2 = [128, 2, TB] fp32 (2 banks) x3 bufs for the
    # two-step mm1 lookahead; p = [128, TB] (1 bank) x2 bufs
    gpool = ctx.enter_context(tc.tile_pool(name="gps", bufs=3, space="PSUM"))
    ppool = ctx.enter_context(tc.tile_pool(name="pps", bufs=2, space="PSUM"))
    # DRAM scratch for the bf16 transpose round-trip
    dpool = ctx.enter_context(tc.tile_pool(name="dsc", bufs=2, space="DRAM"))

    act_credit = 0.0  # fractional round-robin for non-diagonal pair owners

    for b in range(bpc):
        xb_dram = x_ap[b].rearrange("(k p) c -> p k c", p=128)   # [128, nt, C]
        ob_dram = out_ap[b].rearrange("(k p) c -> p k c", p=128)

        # ---- prologue: load, row stats, Y = e_s * x (bf16), X^T dup ----
        x32 = xpool.tile([128, nt, C], FP32)
        nc.sync.dma_start(out=x32[:], in_=xb_dram)

        xx = xxpool.tile([128, nt, C], FP32)
        nc.vector.tensor_mul(xx[:], x32[:], x32[:])
        sq = sqpool.tile([128, nt], FP32, tag="sq")
        nc.vector.tensor_reduce(
            sq[:], xx[:], axis=mybir.AxisListType.X, op=mybir.AluOpType.add
        )
        ev = sqpool.tile([128, nt], FP32, tag="ev")
        nc.scalar.activation(
            ev[:], sq[:], mybir.ActivationFunctionType.Exp, scale=-r
        )
        ev_bc = ev[:, :, None].broadcast_to([128, nt, C])

        yb = ypool.tile([128, nt, C], BF16)
        nc.vector.tensor_mul(yb[:], x32[:], ev_bc)

        # bf16 copy of x; the DMA to DRAM writes it TWICE side by side
        # (cols 0:C and C:2C) so a single full-width DMA-xbar transpose
        # yields X^T duplicated on both partition halves -- mm1 streams two
        # s-blocks concurrently through PE row groups (0,0)/(64,0).
        xb16 = xbpool.tile([128, nt, C], BF16)
        nc.gpsimd.tensor_copy(xb16[:], x32[:])
        xbd = dpool.tile([t, 2 * C], BF16)
        xbd_p = xbd.rearrange("(k p) c -> p k c", p=128)
        nc.sync.dma_start(out=xbd_p[:, :, 0:C], in_=xb16[:])
        nc.sync.dma_start(out=xbd_p[:, :, C : 2 * C], in_=xb16[:])
        xt = xtpool.tile([128, t], BF16)
        nc.sync.dma_start_transpose(out=xt[:], in_=xbd[:])
        # xt[c, tt] = xt[64+c, tt] = x[tt, c] for c < 64.
        if dbg and b == 0:
            dump("dbg_sq", sq[:])
            dump("dbg_ev", ev[:])
            dump("dbg_yb", yb[:])
            dump("dbg_xt", xt[:])

        # ---- main loop: flat (ti, j) steps, mm1 two steps ahead ----
        # otb partitions 0:64 hold the s-even half of out^T; partitions
        # 64:128 the s-odd half.
        otb = otpool.tile([128, t], BF16)

        steps = [(ti, j) for ti in range(ntb) for j in range(npair)]

        owners = []
        for ti, j in steps:
            if j in (2 * ti, 2 * ti + 1):
                owners.append("act")       # diagonal pair: exact exp
            else:
                act_credit += ACT_NONDIAG_FRAC
                if act_credit >= 1.0:
                    act_credit -= 1.0
                    owners.append("act")
                else:
                    owners.append("dve")

        def mm1(step):
            """Concurrent dual row-tile pair: G for s-blocks 2j, 2j+1."""
            ti, j = steps[step]
            g2 = gpool.tile([128, 2, TB], FP32, name="g_ps", tag="g")
            for i in range(2):
                base = 64 * i
                s = 2 * j + i
                nc.tensor.matmul(
                    g2[:, i],
                    lhsT=xt[base : base + 64, s * 128 : (s + 1) * 128],
                    rhs=xt[base : base + 64, ti * TB : (ti + 1) * TB],
                    start=True,
                    stop=True,
                )
            return g2

        p_ps = None
        gq = [mm1(0), mm1(1)]  # two-step lookahead queue
        for step, (ti, j) in enumerate(steps):
            if j == 0:
                p_ps = ppool.tile([128, TB], FP32, tag="p")
            g_cur = gq.pop(0)
            if step + 2 < len(steps):
                gq.append(mm1(step + 2))

            if owners[step] == "act":
                a0t = apool.tile([128, 2, TB], BF16)
                nc.scalar.activation(
                    a0t[:], g_cur[:], mybir.ActivationFunctionType.Exp,
                    scale=exp2r,
                )
                a0 = a0t[:]
            else:
                i16 = ipool.tile([128, 2, TB], I16)
                nc.vector.tensor_scalar(
                    i16[:],
                    g_cur[:],
                    exp2r * SCHRAU_C1,
                    SCHRAU_C2,
                    op0=mybir.AluOpType.mult,
                    op1=mybir.AluOpType.add,
                )
                a0 = i16[:].bitcast(BF16)
            if dbg and b == 0 and step == 0:
                gsb = xxpool.tile([128, 2, TB], FP32, tag="gdump")
                nc.vector.tensor_copy(gsb[:], g_cur[:])
                dump("dbg_g00", gsb[:])

            # mm2: concurrent dual col-tile pair -> partition halves of p_ps
            for i in range(2):
                nc.tensor.matmul(
                    p_ps[64 * i : 64 * i + 64, :],
                    lhsT=yb[:, 2 * j + i],
                    rhs=a0[:, i],
                    start=(j == 0),
                    stop=(j == npair - 1),
                    tile_position=(0, 64 * i),
                    skip_group_check=True,
                )

            if j == npair - 1:
                # single full-width copy; halves stay in their partition
                # ranges.  Alternate ACT/DVE per t-block.
                dst = otb[:, ti * TB : (ti + 1) * TB]
                if ti % 2 == 0:
                    nc.scalar.activation(
                        dst, p_ps[:], mybir.ActivationFunctionType.Copy
                    )
                else:
                    nc.vector.tensor_copy(dst, p_ps[:])

        # ---- epilogue: one whole-batch transpose per half, then
        # out = (trA + trB) * e_t + x via big broadcast-scaled ops ----
        trb = trpool.tile([128, 2, nt, C], BF16)
        nc.sync.dma_start_transpose(out=trb[:, 0], in_=otb[0:64, :])
        nc.sync.dma_start_transpose(out=trb[:, 1], in_=otb[64:128, :])
        if dbg and b == 0:
            dump("dbg_otb", otb[:])
            dump("dbg_trb", trb[:])
        o1 = opool.tile([128, nt, C], FP32, tag="o1")
        o2 = opool.tile([128, nt, C], FP32, tag="o2")
        o3 = opool.tile([128, nt, C], FP32, tag="o3")
        osb = opool.tile([128, nt, C], FP32, tag="osb")
        nc.vector.tensor_mul(o1[:], trb[:, 0], ev_bc)
        nc.gpsimd.tensor_add(o2[:], o1[:], x32[:])
        nc.vector.tensor_mul(o3[:], trb[:, 1], ev_bc)
        nc.vector.tensor_add(osb[:], o3[:], o2[:])
        nc.sync.dma_start(out=ob_dram, in_=osb[:])


def build(r, bpc=BPC, t=T, dbg=False):
    """Build + compile the Bass module for one core's shard."""
    from contextlib import ExitStack

    nc = bacc.Bacc(
        "TRN2", target_bir_lowering=False, debug=False, num_devices=N_CORES
    )
    x_ap = nc.dram_tensor("x", [bpc, t, C], FP32, kind="ExternalInput").ap()
    out_ap = nc.dram_tensor("out", [bpc, t, C], FP32, kind="ExternalOutput").ap()
    with tile.TileContext(nc) as tc:
        with ExitStack() as ctx:
            _body(ctx, tc, out_ap, x_ap, r, bpc, t, dbg=dbg)
    nc.compile()
    return nc


def kernel(x, r_sigma):
    global LAST_RESULTS
    x = np.ascontiguousarray(np.asarray(x, dtype=np.float32))
    r = float(np.asarray(r_sigma).reshape(-1)[0])
    assert x.shape == (B, T, C), x.shape

    nc = build(r)
    in_maps = [
        {"x": np.ascontiguousarray(x[i * BPC : (i + 1) * BPC])}
        for i in range(N_CORES)
    ]
    trace = bool(int(os.environ.get("KERNEL_TRACE", "0")))
    res = run_bass_kernel_spmd(
        nc, in_maps, core_ids=list(range(N_CORES)), trace=trace
    )
    LAST_RESULTS = res
    out = np.concatenate([res.results[i]["out"] for i in range(N_CORES)], axis=0)
    return out.astype(np.float32)
